# revision 15
# baseline (speedup 1.0000x reference)
"""Chamfer-distance loss (CCHLoss) kernel for 8 Trainium2 NeuronCores.

Contract: kernel(**inputs) takes the FULL unsharded inputs
  v:        (16, 2048, 3) f32
  v_pred:   (16, 2048, 3) f32
  mask:     (4, 4, 2, 32, 32) f32
  pred_dw:  (16, 2048, 3) f32
and returns (loss, loss_normals) matching reference().

Strategy: data-parallel over the B=16 batch dim, 2 batches per core.
Per batch the 2048x2048 squared-distance matrix is produced by TensorE
as E = |y|^2 - 2 x.y via a K=11 bf16 hi/lo matmul; the |x|^2 term is
fused into the ACT-engine PSUM->SBUF evacuation as a per-partition
fp32 bias (activation Identity), so ACT emits D = E + |x|^2 in bf16.
i-tiles rotate over the four 32-row PE groups (tile_position derived
from base partitions 0/32/64/96; rhs replicated into the four blocks)
so LDWEIGHTS overlaps the matmul stream.  Reductions are split:
  - VectorE: col-min accumulate (bf16 2x tensor_tensor) + first
    row-min fold per group,
  - GpSimd (Pool): remaining row-min fold tiers for most groups,
  - tail: fold chains + PE transposes + a fused scalar_tensor_tensor
    masked sum.
Host only shards/permutes inputs and sums 8 cores' partial sums.
"""

import numpy as np

B, P1, P2, D = 16, 2048, 2048, 3
NCORES = 8
BPC = B // NCORES  # batches per core
NT = P1 // 128     # i-tiles per batch
NJ = P2 // 512     # matmul j-chunks per group
NC128 = P2 // 128  # 128-wide j-chunks (transpose fold)
KROWS = 11         # lhsT/rhs contraction rows

_CACHE = {}


def build_bass():
    """Build + compile the per-core Bass program (same program all 8 cores)."""
    import concourse.bacc as bacc
    import concourse.tile as tile
    from concourse import mybir
    from concourse.masks import make_identity

    f32 = mybir.dt.float32
    bf16 = mybir.dt.bfloat16
    Alu = mybir.AluOpType
    Act = mybir.ActivationFunctionType
    X = mybir.AxisListType.X

    nc = bacc.Bacc("TRN2", target_bir_lowering=False, debug=False)

    # host-shipped tensors
    xprod_h = nc.dram_tensor("xprod", (BPC, 2, KROWS, 1024), bf16, kind="ExternalInput")
    yprod_h = nc.dram_tensor("yprod", (BPC, KROWS, P2), bf16, kind="ExternalInput")
    cdx_h = nc.dram_tensor("cdx", (BPC, 128, 48), f32, kind="ExternalInput")
    cdy_h = nc.dram_tensor("cdy", (BPC, 128, 48), f32, kind="ExternalInput")
    maskT_h = nc.dram_tensor("maskT", (BPC, 128, NC128), f32, kind="ExternalInput")
    dw_h = nc.dram_tensor("dw", (128, BPC * 48), f32, kind="ExternalInput")
    out_h = nc.dram_tensor("out", (1, 8), f32, kind="ExternalOutput")

    with tile.TileContext(nc) as tc:
        with (
            tc.tile_pool(name="consts", bufs=1) as consts,
            tc.tile_pool(name="opnds", bufs=2) as opnds,
            tc.tile_pool(name="scr", bufs=3) as scr,
            tc.tile_pool(name="small", bufs=4) as small,
            tc.tile_pool(name="ps", bufs=2, space="PSUM") as ps,
        ):
            ones128 = consts.tile([128, 1], f32)
            nc.vector.memset(ones128, 1.0)
            partials = consts.tile([128, 8], f32)
            nc.vector.memset(partials, 0.0)

            # ---------- input DMAs first (all of them) ----------
            # queues: sync = yp input/replica + norm-row writes (critical);
            # scalar = tiny cd inputs then norm rows, NO dmas in steady state;
            # gpsimd (SWDGE) = xp + non-critical loads.
            e_a = nc.sync
            e_b = nc.scalar
            e_g = nc.gpsimd
            cds, yps, xps, mks = [], [], [], []
            for b in range(BPC):
                cdy = opnds.tile([128, 48], f32, tag="cdy")
                e_b.dma_start(out=cdy[:], in_=cdy_h[b])
                cdx = opnds.tile([128, 48], f32, tag="cdx")
                e_b.dma_start(out=cdx[:], in_=cdx_h[b])
                cds.append((cdy, cdx))
            for b in range(BPC):
                yp = opnds.tile([128, P2], bf16, tag="yp")
                e_a.dma_start(out=yp[0:KROWS, :], in_=yprod_h[b])
                # replicate rows 0..8 into block 2 (norm rows come later)
                e_a.dma_start(out=yp[64:64 + 9, :], in_=yp[0:9, :])
                yps.append(yp)
                xp = opnds.tile([128, 1024], bf16, tag="xp")
                for q in range(2):
                    e_g.dma_start(
                        out=xp[64 * q:64 * q + KROWS, :], in_=xprod_h[b, q]
                    )
                xps.append(xp)
                mk = small.tile([128, NC128], f32, tag="mk")
                e_g.dma_start(out=mk[:], in_=maskT_h[b])
                mks.append(mk)
            dwt = consts.tile([128, BPC * 48], f32)
            e_g.dma_start(out=dwt[:], in_=dw_h[:])

            # warm the ACT activation table off the critical path
            warm = consts.tile([1, 1], f32)
            nc.vector.memset(warm, 0.0)
            warmo = consts.tile([1, 1], f32)
            nc.scalar.activation(out=warmo[:], in_=warm[:], func=Act.Square)

            # ---------- per-batch norm prep ----------
            nrmxs = []
            for b in range(BPC):
                cdy, cdx = cds[b]
                yp = yps[b]
                # y norms: bf16 hi/lo rows into both blocks
                sqy = opnds.tile([128, 48], f32, tag="sqy")
                nc.scalar.activation(out=sqy[:], in_=cdy[:], func=Act.Square)
                nrmy = opnds.tile([128, 16], f32, tag="nrmy")
                nc.vector.tensor_reduce(
                    out=nrmy[:], in_=sqy[:].rearrange("p (n d) -> p n d", d=3),
                    axis=X, op=Alu.add,
                )
                nyh = opnds.tile([128, 16], bf16, tag="nyh")
                nc.scalar.copy(nyh[:], nrmy[:])
                nyl = opnds.tile([128, 16], bf16, tag="nyl")
                nc.vector.tensor_tensor(
                    out=nyl[:], in0=nrmy[:], in1=nyh[:], op=Alu.subtract
                )
                for q in range(2):
                    e_b.dma_start(out=yp[64 * q + 9:64 * q + 10, :], in_=nyh[:])
                    e_b.dma_start(out=yp[64 * q + 10:64 * q + 11, :], in_=nyl[:])
                # x norms: stay fp32 in SBUF, consumed as ACT bias
                sqx = opnds.tile([128, 48], f32, tag="sqx")
                nc.scalar.activation(out=sqx[:], in_=cdx[:], func=Act.Square)
                nrmx = opnds.tile([128, 16], f32, tag="nrmx")
                nc.vector.tensor_reduce(
                    out=nrmx[:], in_=sqx[:].rearrange("p (n d) -> p n d", d=3),
                    axis=X, op=Alu.add,
                )
                nrmxs.append(nrmx)

            # --- mean(pred_dw^2) partial: ACT square with sum-accumulate ---
            dwsq = consts.tile([128, BPC * 48], f32)
            nc.scalar.activation(
                out=dwsq[:], in_=dwt[:], func=Act.Square,
                accum_out=partials[:, 6:7],
            )

            ident = consts.tile([128, 128], bf16)
            make_identity(nc, ident)

            # ---------- main distance + min pipeline ----------
            colaccs = [
                opnds.tile([128, P2], bf16, tag="colacc", name=f"colacc{b}")
                for b in range(BPC)
            ]
            rowpartss = [
                opnds.tile([128, NT, 128], bf16, tag="rowparts",
                           name=f"rowparts{b}")
                for b in range(BPC)
            ]

            def emit_seg(b, t0, n):
                """n (2 or 4) i-tiles [t0, t0+n): matmuls + evac + DVE mins."""
                xp, yp, nrmx = xps[b], yps[b], nrmxs[b]
                colacc, rowparts = colaccs[b], rowpartss[b]
                s4 = scr.tile([128, n, P2], bf16, tag=f"s4_{n}", bufs=2)
                for u in range(n):
                    t = t0 + u
                    # PE row block alternates per pair; rank = col chunk
                    qb = 64 * ((t >> 1) & 1)
                    r = (t >> 2) * 2 + (t & 1)
                    lsl = xp[qb:qb + KROWS, 128 * r:128 * (r + 1)]
                    g = ps.tile([128, P2], f32, tag="dgrp")
                    for c in range(NJ):
                        sl = slice(c * 512, (c + 1) * 512)
                        nc.tensor.matmul(
                            g[:, sl], lsl, yp[qb:qb + KROWS, sl],
                            tile_position=(qb, 0),
                        )
                    # ACT evacuation: D = E + |x|^2 (partition bias), bf16
                    nc.scalar.activation(
                        out=s4[:, u, :], in_=g[:], func=Act.Identity,
                        bias=nrmx[:, t:t + 1],
                    )
                # row-min fold chain, seg-batched (DVE bf16 2x)
                src = s4[:]
                w = P2
                while w > 256:
                    w //= 2
                    dst = scr.tile([128, n, w], bf16, tag=f"rp{w}_{n}", bufs=2)
                    nc.vector.tensor_tensor(
                        out=dst[:], in0=src[:, :, 0:w], in1=src[:, :, w:2 * w],
                        op=Alu.min,
                    )
                    src = dst[:]
                nc.vector.tensor_tensor(
                    out=rowparts[:, t0:t0 + n, :],
                    in0=src[:, :, 0:128], in1=src[:, :, 128:256], op=Alu.min,
                )
                # col-min accumulate, seg-batched
                if n == 4:
                    m2 = scr.tile([128, 2, P2], bf16, tag="m2", bufs=2)
                    nc.vector.tensor_tensor(
                        out=m2[:], in0=s4[:, 0:2, :], in1=s4[:, 2:4, :],
                        op=Alu.min,
                    )
                    lo, hi = m2[:, 0, :], m2[:, 1, :]
                else:
                    lo, hi = s4[:, 0, :], s4[:, 1, :]
                if t0 == 0:
                    nc.vector.tensor_tensor(
                        out=colacc[:], in0=lo, in1=hi, op=Alu.min
                    )
                else:
                    mmp = scr.tile([128, P2], bf16, tag="mmp", bufs=2)
                    nc.vector.tensor_tensor(out=mmp[:], in0=lo, in1=hi,
                                            op=Alu.min)
                    nc.vector.tensor_tensor(
                        out=colacc[:], in0=colacc[:], in1=mmp[:], op=Alu.min
                    )

            def emit_tail(b):
                """Row-min finish + cham_v transposes + masked sums."""
                colacc, rowparts, mk = colaccs[b], rowpartss[b], mks[b]
                # finish row-min: [128,16,128] -> [128,16,1] fold chain
                w = 128
                src = rowparts[:]
                while w > 1:
                    w //= 2
                    dst = small.tile([128, NT, w], bf16, tag=f"rw{w}")
                    nc.vector.tensor_tensor(
                        out=dst[:], in0=src[:, :, 0:w], in1=src[:, :, w:2 * w],
                        op=Alu.min,
                    )
                    src = dst[:]
                rowmin16 = src  # [128, 16, 1]
                nc.vector.tensor_reduce(
                    out=partials[:, 2 * b + 1:2 * b + 2],
                    in_=rowmin16.rearrange("p a b -> p (a b)"),
                    axis=X, op=Alu.add,
                )
                # cham_v: fold colacc partitions via PE transpose
                chamv = small.tile([128, NC128], f32, tag="chamv")
                for r in range(2):
                    tp = ps.tile([128, 1024], bf16, tag="dgrp")
                    for cc in range(8):
                        cidx = r * 8 + cc
                        nc.tensor.transpose(
                            tp[:, cc * 128:(cc + 1) * 128],
                            colacc[:, cidx * 128:(cidx + 1) * 128],
                            ident[:],
                        )
                    nc.vector.tensor_reduce(
                        out=chamv[:, r * 8:(r + 1) * 8],
                        in_=tp[:].rearrange("p (a c) -> p a c", c=128),
                        axis=X, op=Alu.min,
                    )
                # masked sum: out = (chamv + 0) * mask, accum = sum
                prod = small.tile([128, NC128], f32, tag="prod")
                nc.vector.scalar_tensor_tensor(
                    out=prod[:], in0=chamv[:], scalar=0.0, in1=mk[:],
                    op0=Alu.add, op1=Alu.mult,
                    accum_out=partials[:, 2 * b:2 * b + 1],
                )

            # batch 0: two warm-up pairs then quads; batch-0 tail is emitted
            # after batch 1's first quad so it fills pipeline gaps instead of
            # serializing the batch hand-off.
            for t0 in (0, 2):
                emit_seg(0, t0, 2)
            for t0 in (4, 8, 12):
                emit_seg(0, t0, 4)
            emit_seg(1, 0, 4)
            emit_tail(0)
            for t0 in (4, 8, 12):
                emit_seg(1, t0, 4)
            emit_tail(1)

            # ---------- cross-partition sum of all partials via PE ----------
            fin = ps.tile([128, 512], f32, tag="dgrp")
            nc.tensor.matmul(fin[0:1, 0:8], ones128[:], partials[:])
            res = small.tile([1, 8], f32, tag="res")
            nc.scalar.copy(res[:], fin[0:1, 0:8])
            nc.sync.dma_start(out=out_h[:], in_=res[:])

    nc.compile()
    return nc


def get_compiled():
    if "nc" not in _CACHE:
        _CACHE["nc"] = build_bass()
    return _CACHE["nc"]


def make_in_maps(v, v_pred, mask, pred_dw):
    import ml_dtypes

    bf16 = ml_dtypes.bfloat16
    v = np.asarray(v, np.float32)
    v_pred = np.asarray(v_pred, np.float32)
    mask = np.asarray(mask, np.float32)
    pred_dw = np.asarray(pred_dw, np.float32)

    # lossless bf16 hi/lo repacking of the matmul operands
    wT = (-2.0 * v_pred).transpose(0, 2, 1)           # (16, 3, 2048) f32
    wh = wT.astype(bf16)
    wl = (wT - wh.astype(np.float32)).astype(bf16)
    ones = np.ones((B, 2, P1), dtype=bf16)
    # lhsT rows: [wh x3, wl x3, wh x3, 1, 1]  (K=11)
    xprod = np.concatenate([wh, wl, wh, ones], axis=1)      # (16, 11, 2048)
    # PE row block q' = (t>>1)&1, rank r = (t>>2)*2 + (t&1):
    # xprod_dev[b, q', :, 128r:128(r+1)] = xprod[b, :, 128t:128(t+1)]
    # column c = 128t + j with t = 4a + 2b2 + b1 -> (a, b2, b1, j);
    # block = b2, rank = 2a + b1.
    xprod_dev = np.ascontiguousarray(
        xprod.reshape(B, KROWS, 4, 2, 2, 128).transpose(0, 3, 1, 2, 4, 5)
        .reshape(B, 2, KROWS, 1024)
    )
    yT = v.transpose(0, 2, 1)
    yh = yT.astype(bf16)
    yl = (yT - yh.astype(np.float32)).astype(bf16)
    zeros2 = np.zeros((B, 2, P2), dtype=bf16)
    # rhs rows: [yh x3, yh x3, yl x3, <y2 hi/lo slots>]  (K=11)
    yprod = np.concatenate([yh, yh, yl, zeros2], axis=1)    # (16, 11, 2048)

    # cdx permuted so nrmx[p, t] = |x_{128t+p}|^2
    cdx = np.ascontiguousarray(
        v_pred.reshape(B, 16, 128, 3).transpose(0, 2, 1, 3).reshape(B, 128, 48)
    )
    # cdy as-is: nrmy[p, n] = |y_{16p+n}|^2 -> row-major j ordering
    cdy = v.reshape(B, 128, 48)
    mask_flat = mask.reshape(B, P2)
    # maskT[b, p, c] = mask_flat[b, c*128 + p]
    maskT = np.ascontiguousarray(
        mask_flat.reshape(B, NC128, 128).transpose(0, 2, 1)
    )
    in_maps = []
    for kcore in range(NCORES):
        b0 = BPC * kcore
        dwp = np.concatenate(
            [pred_dw[b0 + i].reshape(128, 48) for i in range(BPC)], axis=1
        )
        in_maps.append({
            "xprod": np.ascontiguousarray(xprod_dev[b0:b0 + BPC]),
            "yprod": np.ascontiguousarray(yprod[b0:b0 + BPC]),
            "cdx": np.ascontiguousarray(cdx[b0:b0 + BPC]),
            "cdy": np.ascontiguousarray(cdy[b0:b0 + BPC]),
            "maskT": np.ascontiguousarray(maskT[b0:b0 + BPC]),
            "dw": np.ascontiguousarray(dwp),
        })
    return in_maps


def combine_outs(outs):
    """outs: (8, 8) array of per-core partial rows -> (loss, loss_normals)."""
    outs = np.asarray(outs, np.float64)
    mcols = [2 * i for i in range(BPC)]
    rcols = [2 * i + 1 for i in range(BPC)]
    msum = outs[:, mcols].sum()
    rsum = outs[:, rcols].sum()
    dsum = outs[:, 6].sum()
    loss = msum / (B * P2) + rsum / (B * P1) + dsum / (B * P1 * D)
    return (np.float32(loss), np.float32(0.0))


def kernel(**inputs):
    from concourse.bass_utils import run_bass_kernel_spmd

    nc = get_compiled()
    in_maps = make_in_maps(
        inputs["v"], inputs["v_pred"], inputs["mask"], inputs["pred_dw"]
    )
    res = run_bass_kernel_spmd(nc, in_maps, core_ids=list(range(NCORES)))
    outs = np.stack([r["out"].reshape(8) for r in res.results])
    return combine_outs(outs)


# revision 19
# speedup vs baseline: 1.0616x; 1.0616x over previous
"""Chamfer-distance loss (CCHLoss) kernel for 8 Trainium2 NeuronCores.

Contract: kernel(**inputs) takes the FULL unsharded inputs
  v:        (16, 2048, 3) f32
  v_pred:   (16, 2048, 3) f32
  mask:     (4, 4, 2, 32, 32) f32
  pred_dw:  (16, 2048, 3) f32
and returns (loss, loss_normals) matching reference().

Strategy: data-parallel over the B=16 batch dim, 2 batches per core.
Per batch the 2048x2048 squared-distance matrix is produced by TensorE
as E = |y|^2 - 2 x.y via a K=11 bf16 hi/lo matmul; the |x|^2 term is
fused into the ACT-engine PSUM->SBUF evacuation as a per-partition
fp32 bias (activation Identity), so ACT emits D = E + |x|^2 in bf16.
i-tiles rotate over the four 32-row PE groups (tile_position derived
from base partitions 0/32/64/96; rhs replicated into the four blocks)
so LDWEIGHTS overlaps the matmul stream.  Reductions are split:
  - VectorE: col-min accumulate (bf16 2x tensor_tensor) + first
    row-min fold per group,
  - GpSimd (Pool): remaining row-min fold tiers for most groups,
  - tail: fold chains + PE transposes + a fused scalar_tensor_tensor
    masked sum.
Host only shards/permutes inputs and sums 8 cores' partial sums.
"""

import numpy as np

B, P1, P2, D = 16, 2048, 2048, 3
NCORES = 8
BPC = B // NCORES  # batches per core
NT = P1 // 128     # i-tiles per batch
NJ = P2 // 512     # matmul j-chunks per group
NC128 = P2 // 128  # 128-wide j-chunks (transpose fold)
KROWS = 11         # lhsT/rhs contraction rows

_CACHE = {}


def build_bass():
    """Build + compile the per-core Bass program (same program all 8 cores)."""
    import concourse.bacc as bacc
    import concourse.tile as tile
    from concourse import mybir
    from concourse.masks import make_identity

    f32 = mybir.dt.float32
    bf16 = mybir.dt.bfloat16
    Alu = mybir.AluOpType
    Act = mybir.ActivationFunctionType
    X = mybir.AxisListType.X

    nc = bacc.Bacc("TRN2", target_bir_lowering=False, debug=False)

    # host-shipped tensors
    xprod_h = nc.dram_tensor("xprod", (BPC, 2, KROWS, 1024), bf16, kind="ExternalInput")
    yprod_h = nc.dram_tensor("yprod", (BPC, KROWS, P2), bf16, kind="ExternalInput")
    cdx_h = nc.dram_tensor("cdx", (BPC, 128, 48), f32, kind="ExternalInput")
    cdy_h = nc.dram_tensor("cdy", (BPC, 128, 48), f32, kind="ExternalInput")
    maskT_h = nc.dram_tensor("maskT", (BPC, 128, NC128), f32, kind="ExternalInput")
    dw_h = nc.dram_tensor("dw", (128, BPC * 48), f32, kind="ExternalInput")
    out_h = nc.dram_tensor("out", (1, 8), f32, kind="ExternalOutput")

    with tile.TileContext(nc) as tc:
        with (
            tc.tile_pool(name="consts", bufs=1) as consts,
            tc.tile_pool(name="opnds", bufs=2) as opnds,
            tc.tile_pool(name="scr", bufs=3) as scr,
            tc.tile_pool(name="small", bufs=4) as small,
            tc.tile_pool(name="ps", bufs=2, space="PSUM") as ps,
        ):
            ones128 = consts.tile([128, 1], f32)
            nc.vector.memset(ones128, 1.0)
            partials = consts.tile([128, 8], f32)
            nc.vector.memset(partials, 0.0)

            # ---------- input DMAs first (all of them) ----------
            # queues: sync = yp input/replica + norm-row writes (critical);
            # scalar = tiny cd inputs then norm rows, NO dmas in steady state;
            # gpsimd (SWDGE) = xp + non-critical loads.
            e_a = nc.sync
            e_b = nc.scalar
            e_g = nc.gpsimd
            cds, yps, xps, mks = [], [], [], []
            for b in range(BPC):
                cd = opnds.tile([128, 96], f32, tag="cd")
                e_b.dma_start(out=cd[:, 0:48], in_=cdy_h[b])
                e_b.dma_start(out=cd[:, 48:96], in_=cdx_h[b])
                cds.append(cd)
            for b in range(BPC):
                yp = opnds.tile([128, P2], bf16, tag="yp")
                e_a.dma_start(out=yp[0:KROWS, :], in_=yprod_h[b])
                # replicate rows 0..8 into block 2 (norm rows come later)
                e_a.dma_start(out=yp[64:64 + 9, :], in_=yp[0:9, :])
                yps.append(yp)
                xp = opnds.tile([128, 1024], bf16, tag="xp")
                for q in range(2):
                    e_g.dma_start(
                        out=xp[64 * q:64 * q + KROWS, :], in_=xprod_h[b, q]
                    )
                xps.append(xp)
                mk = small.tile([128, NC128], f32, tag="mk")
                e_g.dma_start(out=mk[:], in_=maskT_h[b])
                mks.append(mk)
            dwt = consts.tile([128, BPC * 48], f32)
            e_g.dma_start(out=dwt[:], in_=dw_h[:])

            # warm the ACT activation table off the critical path
            warm = consts.tile([1, 1], f32)
            nc.vector.memset(warm, 0.0)
            warmo = consts.tile([1, 1], f32)
            nc.scalar.activation(out=warmo[:], in_=warm[:], func=Act.Square)

            # ---------- per-batch norm prep ----------
            # one square + one reduce covers both y (cols 0:16) and x (16:32)
            nrmxs = []
            for b in range(BPC):
                cd = cds[b]
                yp = yps[b]
                sq = opnds.tile([128, 96], f32, tag="sq")
                nc.scalar.activation(out=sq[:], in_=cd[:], func=Act.Square)
                nrm = opnds.tile([128, 32], f32, tag="nrm")
                nc.vector.tensor_reduce(
                    out=nrm[:], in_=sq[:].rearrange("p (n d) -> p n d", d=3),
                    axis=X, op=Alu.add,
                )
                # y norms: bf16 hi/lo rows into both blocks (DVE-only chain)
                nyh = opnds.tile([128, 16], bf16, tag="nyh")
                nc.vector.tensor_copy(out=nyh[:], in_=nrm[:, 0:16])
                nyl = opnds.tile([128, 16], bf16, tag="nyl")
                nc.vector.tensor_tensor(
                    out=nyl[:], in0=nrm[:, 0:16], in1=nyh[:], op=Alu.subtract
                )
                for q in range(2):
                    e_b.dma_start(out=yp[64 * q + 9:64 * q + 10, :], in_=nyh[:])
                    e_b.dma_start(out=yp[64 * q + 10:64 * q + 11, :], in_=nyl[:])
                # x norms: nrm[:, 16:32] stays fp32, consumed as ACT bias
                nrmxs.append(nrm[:, 16:32])

            # --- mean(pred_dw^2) partial: ACT square with sum-accumulate ---
            dwsq = consts.tile([128, BPC * 48], f32)
            nc.scalar.activation(
                out=dwsq[:], in_=dwt[:], func=Act.Square,
                accum_out=partials[:, 6:7],
            )

            ident = consts.tile([128, 128], bf16)
            make_identity(nc, ident)

            # ---------- main distance + min pipeline ----------
            colaccs = [
                opnds.tile([128, P2], bf16, tag="colacc", name=f"colacc{b}")
                for b in range(BPC)
            ]
            rowpartss = [
                opnds.tile([128, NT, 128], bf16, tag="rowparts",
                           name=f"rowparts{b}")
                for b in range(BPC)
            ]

            def emit_seg(b, t0, n):
                """n (2 or 4) i-tiles [t0, t0+n): matmuls + evac + DVE mins."""
                xp, yp, nrmx = xps[b], yps[b], nrmxs[b]
                colacc, rowparts = colaccs[b], rowpartss[b]
                s4 = scr.tile([128, n, P2], bf16, tag=f"s4_{n}", bufs=3)
                for u in range(n):
                    t = t0 + u
                    # PE row block alternates per pair; rank = col chunk
                    qb = 64 * ((t >> 1) & 1)
                    r = (t >> 2) * 2 + (t & 1)
                    lsl = xp[qb:qb + KROWS, 128 * r:128 * (r + 1)]
                    g = ps.tile([128, P2], f32, tag="dgrp")
                    for c in range(NJ):
                        sl = slice(c * 512, (c + 1) * 512)
                        nc.tensor.matmul(
                            g[:, sl], lsl, yp[qb:qb + KROWS, sl],
                            tile_position=(qb, 0),
                        )
                    # ACT evacuation: D = E + |x|^2 (partition bias), bf16
                    nc.scalar.activation(
                        out=s4[:, u, :], in_=g[:], func=Act.Identity,
                        bias=nrmx[:, t:t + 1],
                    )
                # row-min fold chain, seg-batched (DVE bf16 2x)
                src = s4[:]
                w = P2
                while w > 256:
                    w //= 2
                    dst = scr.tile([128, n, w], bf16, tag=f"rp{w}_{n}", bufs=2)
                    nc.vector.tensor_tensor(
                        out=dst[:], in0=src[:, :, 0:w], in1=src[:, :, w:2 * w],
                        op=Alu.min,
                    )
                    src = dst[:]
                nc.vector.tensor_tensor(
                    out=rowparts[:, t0:t0 + n, :],
                    in0=src[:, :, 0:128], in1=src[:, :, 128:256], op=Alu.min,
                )
                # col-min accumulate, seg-batched
                if n == 4:
                    m2 = scr.tile([128, 2, P2], bf16, tag="m2", bufs=2)
                    nc.vector.tensor_tensor(
                        out=m2[:], in0=s4[:, 0:2, :], in1=s4[:, 2:4, :],
                        op=Alu.min,
                    )
                    lo, hi = m2[:, 0, :], m2[:, 1, :]
                else:
                    lo, hi = s4[:, 0, :], s4[:, 1, :]
                if t0 == 0:
                    nc.vector.tensor_tensor(
                        out=colacc[:], in0=lo, in1=hi, op=Alu.min
                    )
                else:
                    mmp = scr.tile([128, P2], bf16, tag="mmp", bufs=2)
                    nc.vector.tensor_tensor(out=mmp[:], in0=lo, in1=hi,
                                            op=Alu.min)
                    nc.vector.tensor_tensor(
                        out=colacc[:], in0=colacc[:], in1=mmp[:], op=Alu.min
                    )

            def emit_tail(b):
                """Row-min finish + cham_v transposes + masked sums."""
                colacc, rowparts, mk = colaccs[b], rowpartss[b], mks[b]
                # finish row-min: [128,16,128] -> [128,16] min, then sum
                rowacc = small.tile([128, NT], f32, tag="rowacc")
                nc.vector.tensor_reduce(
                    out=rowacc[:], in_=rowparts[:], axis=X, op=Alu.min,
                )
                nc.vector.tensor_reduce(
                    out=partials[:, 2 * b + 1:2 * b + 2], in_=rowacc[:],
                    axis=X, op=Alu.add,
                )
                # cham_v: fold colacc partitions via PE transpose; ACT
                # evacuates the bf16 transpose out of PSUM (contiguous read)
                # so the DVE reduce runs on SBUF without PSUM seg penalties.
                chamv = small.tile([128, NC128], f32, tag="chamv")
                for r in range(2):
                    tp = ps.tile([128, 1024], bf16, tag="dgrp")
                    for cc in range(8):
                        cidx = r * 8 + cc
                        nc.tensor.transpose(
                            tp[:, cc * 128:(cc + 1) * 128],
                            colacc[:, cidx * 128:(cidx + 1) * 128],
                            ident[:],
                        )
                    tps = small.tile([128, 1024], bf16, tag="tps", bufs=2)
                    nc.scalar.copy(tps[:], tp[:])
                    nc.vector.tensor_reduce(
                        out=chamv[:, r * 8:(r + 1) * 8],
                        in_=tps[:].rearrange("p (a c) -> p a c", c=128),
                        axis=X, op=Alu.min,
                    )
                # masked sum: out = (chamv + 0) * mask, accum = sum
                prod = small.tile([128, NC128], f32, tag="prod")
                nc.vector.scalar_tensor_tensor(
                    out=prod[:], in0=chamv[:], scalar=0.0, in1=mk[:],
                    op0=Alu.add, op1=Alu.mult,
                    accum_out=partials[:, 2 * b:2 * b + 1],
                )

            # batch 0: two warm-up pairs then quads; batch-0 tail is emitted
            # after batch 1's first quad so it fills pipeline gaps instead of
            # serializing the batch hand-off.
            for t0 in (0, 2):
                emit_seg(0, t0, 2)
            for t0 in (4, 8, 12):
                emit_seg(0, t0, 4)
            emit_seg(1, 0, 4)
            emit_tail(0)
            for t0 in (4, 8, 12):
                emit_seg(1, t0, 4)
            emit_tail(1)

            # ---------- cross-partition sum of all partials via PE ----------
            fin = ps.tile([128, 512], f32, tag="dgrp")
            nc.tensor.matmul(fin[0:1, 0:8], ones128[:], partials[:])
            res = small.tile([1, 8], f32, tag="res")
            nc.scalar.copy(res[:], fin[0:1, 0:8])
            nc.sync.dma_start(out=out_h[:], in_=res[:])

    nc.compile()
    return nc


def get_compiled():
    if "nc" not in _CACHE:
        _CACHE["nc"] = build_bass()
    return _CACHE["nc"]


def make_in_maps(v, v_pred, mask, pred_dw):
    import ml_dtypes

    bf16 = ml_dtypes.bfloat16
    v = np.asarray(v, np.float32)
    v_pred = np.asarray(v_pred, np.float32)
    mask = np.asarray(mask, np.float32)
    pred_dw = np.asarray(pred_dw, np.float32)

    # lossless bf16 hi/lo repacking of the matmul operands
    wT = (-2.0 * v_pred).transpose(0, 2, 1)           # (16, 3, 2048) f32
    wh = wT.astype(bf16)
    wl = (wT - wh.astype(np.float32)).astype(bf16)
    ones = np.ones((B, 2, P1), dtype=bf16)
    # lhsT rows: [wh x3, wl x3, wh x3, 1, 1]  (K=11)
    xprod = np.concatenate([wh, wl, wh, ones], axis=1)      # (16, 11, 2048)
    # PE row block q' = (t>>1)&1, rank r = (t>>2)*2 + (t&1):
    # xprod_dev[b, q', :, 128r:128(r+1)] = xprod[b, :, 128t:128(t+1)]
    # column c = 128t + j with t = 4a + 2b2 + b1 -> (a, b2, b1, j);
    # block = b2, rank = 2a + b1.
    xprod_dev = np.ascontiguousarray(
        xprod.reshape(B, KROWS, 4, 2, 2, 128).transpose(0, 3, 1, 2, 4, 5)
        .reshape(B, 2, KROWS, 1024)
    )
    yT = v.transpose(0, 2, 1)
    yh = yT.astype(bf16)
    yl = (yT - yh.astype(np.float32)).astype(bf16)
    zeros2 = np.zeros((B, 2, P2), dtype=bf16)
    # rhs rows: [yh x3, yh x3, yl x3, <y2 hi/lo slots>]  (K=11)
    yprod = np.concatenate([yh, yh, yl, zeros2], axis=1)    # (16, 11, 2048)

    # cdx permuted so nrmx[p, t] = |x_{128t+p}|^2
    cdx = np.ascontiguousarray(
        v_pred.reshape(B, 16, 128, 3).transpose(0, 2, 1, 3).reshape(B, 128, 48)
    )
    # cdy as-is: nrmy[p, n] = |y_{16p+n}|^2 -> row-major j ordering
    cdy = v.reshape(B, 128, 48)
    mask_flat = mask.reshape(B, P2)
    # maskT[b, p, c] = mask_flat[b, c*128 + p]
    maskT = np.ascontiguousarray(
        mask_flat.reshape(B, NC128, 128).transpose(0, 2, 1)
    )
    in_maps = []
    for kcore in range(NCORES):
        b0 = BPC * kcore
        dwp = np.concatenate(
            [pred_dw[b0 + i].reshape(128, 48) for i in range(BPC)], axis=1
        )
        in_maps.append({
            "xprod": np.ascontiguousarray(xprod_dev[b0:b0 + BPC]),
            "yprod": np.ascontiguousarray(yprod[b0:b0 + BPC]),
            "cdx": np.ascontiguousarray(cdx[b0:b0 + BPC]),
            "cdy": np.ascontiguousarray(cdy[b0:b0 + BPC]),
            "maskT": np.ascontiguousarray(maskT[b0:b0 + BPC]),
            "dw": np.ascontiguousarray(dwp),
        })
    return in_maps


def combine_outs(outs):
    """outs: (8, 8) array of per-core partial rows -> (loss, loss_normals)."""
    outs = np.asarray(outs, np.float64)
    mcols = [2 * i for i in range(BPC)]
    rcols = [2 * i + 1 for i in range(BPC)]
    msum = outs[:, mcols].sum()
    rsum = outs[:, rcols].sum()
    dsum = outs[:, 6].sum()
    loss = msum / (B * P2) + rsum / (B * P1) + dsum / (B * P1 * D)
    return (np.float32(loss), np.float32(0.0))


def kernel(**inputs):
    from concourse.bass_utils import run_bass_kernel_spmd

    nc = get_compiled()
    in_maps = make_in_maps(
        inputs["v"], inputs["v_pred"], inputs["mask"], inputs["pred_dw"]
    )
    res = run_bass_kernel_spmd(nc, in_maps, core_ids=list(range(NCORES)))
    outs = np.stack([r["out"].reshape(8) for r in res.results])
    return combine_outs(outs)


# revision 25
# speedup vs baseline: 1.0647x; 1.0029x over previous
"""Chamfer-distance loss (CCHLoss) kernel for 8 Trainium2 NeuronCores.

Contract: kernel(**inputs) takes the FULL unsharded inputs
  v:        (16, 2048, 3) f32
  v_pred:   (16, 2048, 3) f32
  mask:     (4, 4, 2, 32, 32) f32
  pred_dw:  (16, 2048, 3) f32
and returns (loss, loss_normals) matching reference().

Strategy: data-parallel over the B=16 batch dim, 2 batches per core.
Per batch the 2048x2048 squared-distance matrix is produced by TensorE
as E = |y|^2 - 2 x.y via a K=11 bf16 hi/lo matmul; the |x|^2 term is
fused into the ACT-engine PSUM->SBUF evacuation as a per-partition
fp32 bias (activation Identity), so ACT emits D = E + |x|^2 in bf16.
i-tiles rotate over the four 32-row PE groups (tile_position derived
from base partitions 0/32/64/96; rhs replicated into the four blocks)
so LDWEIGHTS overlaps the matmul stream.  Reductions are split:
  - VectorE: col-min accumulate (bf16 2x tensor_tensor) + first
    row-min fold per group,
  - GpSimd (Pool): remaining row-min fold tiers for most groups,
  - tail: fold chains + PE transposes + a fused scalar_tensor_tensor
    masked sum.
Host only shards/permutes inputs and sums 8 cores' partial sums.
"""

import numpy as np

B, P1, P2, D = 16, 2048, 2048, 3
NCORES = 8
BPC = B // NCORES  # batches per core
NT = P1 // 128     # i-tiles per batch
NJ = P2 // 512     # matmul j-chunks per group
NC128 = P2 // 128  # 128-wide j-chunks (transpose fold)
KROWS = 11         # lhsT/rhs contraction rows

_CACHE = {}


def build_bass():
    """Build + compile the per-core Bass program (same program all 8 cores)."""
    import concourse.bacc as bacc
    import concourse.tile as tile
    from concourse import mybir
    from concourse.masks import make_identity

    f32 = mybir.dt.float32
    bf16 = mybir.dt.bfloat16
    Alu = mybir.AluOpType
    Act = mybir.ActivationFunctionType
    X = mybir.AxisListType.X

    nc = bacc.Bacc("TRN2", target_bir_lowering=False, debug=False)

    # host-shipped tensors
    xprod_h = nc.dram_tensor("xprod", (BPC, 2, KROWS, 1024), bf16, kind="ExternalInput")
    yprod_h = nc.dram_tensor("yprod", (BPC, KROWS, P2), bf16, kind="ExternalInput")
    cdx_h = nc.dram_tensor("cdx", (BPC, 128, 48), f32, kind="ExternalInput")
    cdy_h = nc.dram_tensor("cdy", (BPC, 128, 48), f32, kind="ExternalInput")
    maskT_h = nc.dram_tensor("maskT", (BPC, 128, NC128), f32, kind="ExternalInput")
    dw_h = nc.dram_tensor("dw", (128, BPC * 48), f32, kind="ExternalInput")
    out_h = nc.dram_tensor("out", (1, 8), f32, kind="ExternalOutput")

    with tile.TileContext(nc) as tc:
        with (
            tc.tile_pool(name="consts", bufs=1) as consts,
            tc.tile_pool(name="opnds", bufs=2) as opnds,
            tc.tile_pool(name="scr", bufs=3) as scr,
            tc.tile_pool(name="small", bufs=4) as small,
            tc.tile_pool(name="ps", bufs=2, space="PSUM") as ps,
        ):
            ones128 = consts.tile([128, 1], f32)
            nc.vector.memset(ones128, 1.0)
            partials = consts.tile([128, 8], f32)
            nc.vector.memset(partials, 0.0)

            # ---------- input DMAs first (all of them) ----------
            # queues: sync = yp input/replica + norm-row writes (critical);
            # scalar = tiny cd inputs then norm rows, NO dmas in steady state;
            # gpsimd (SWDGE) = xp + non-critical loads.
            e_a = nc.sync
            e_b = nc.scalar
            e_g = nc.gpsimd
            cds, yps, xps, mks = [], [], [], []
            for b in range(BPC):
                cd = opnds.tile([128, 96], f32, tag="cd")
                e_b.dma_start(out=cd[:, 0:48], in_=cdy_h[b])
                e_b.dma_start(out=cd[:, 48:96], in_=cdx_h[b])
                cds.append(cd)
            for b in range(BPC):
                yp = opnds.tile([128, P2], bf16, tag="yp")
                e_a.dma_start(out=yp[0:KROWS, :], in_=yprod_h[b])
                # replicate rows 0..8 into block 2 (norm rows come later)
                e_a.dma_start(out=yp[64:64 + 9, :], in_=yp[0:9, :])
                yps.append(yp)
                xp = opnds.tile([128, 1024], bf16, tag="xp")
                for q in range(2):
                    e_g.dma_start(
                        out=xp[64 * q:64 * q + KROWS, :], in_=xprod_h[b, q]
                    )
                xps.append(xp)
                mk = small.tile([128, NC128], f32, tag="mk")
                e_g.dma_start(out=mk[:], in_=maskT_h[b])
                mks.append(mk)
            dwt = consts.tile([128, BPC * 48], f32)
            e_g.dma_start(out=dwt[:], in_=dw_h[:])

            # warm the ACT activation table off the critical path
            warm = consts.tile([1, 1], f32)
            nc.vector.memset(warm, 0.0)
            warmo = consts.tile([1, 1], f32)
            nc.scalar.activation(out=warmo[:], in_=warm[:], func=Act.Square)

            # ---------- per-batch norm prep ----------
            # one square + one reduce covers both y (cols 0:16) and x (16:32)
            nrmxs = []
            nyh0 = nyl0 = None
            for b in range(BPC):
                cd = cds[b]
                yp = yps[b]
                sq = opnds.tile([128, 96], f32, tag="sq")
                nc.scalar.activation(out=sq[:], in_=cd[:], func=Act.Square)
                nrm = opnds.tile([128, 32], f32, tag="nrm")
                nc.vector.tensor_reduce(
                    out=nrm[:], in_=sq[:].rearrange("p (n d) -> p n d", d=3),
                    axis=X, op=Alu.add,
                )
                # y norms: bf16 hi/lo rows into both blocks (DVE-only chain)
                nyh = opnds.tile([128, 16], bf16, tag="nyh")
                nc.vector.tensor_copy(out=nyh[:], in_=nrm[:, 0:16])
                nyl = opnds.tile([128, 16], bf16, tag="nyl")
                nc.vector.tensor_tensor(
                    out=nyl[:], in0=nrm[:, 0:16], in1=nyh[:], op=Alu.subtract
                )
                for q in range(2):
                    e_b.dma_start(out=yp[64 * q + 9:64 * q + 10, :], in_=nyh[:])
                    e_b.dma_start(out=yp[64 * q + 10:64 * q + 11, :], in_=nyl[:])
                # x norms: nrm[:, 16:32] stays fp32, consumed as ACT bias
                nrmxs.append(nrm[:, 16:32])
                if b == 0:
                    nyh0, nyl0 = nyh, nyl

            # --- mean(pred_dw^2) partial: ACT square with sum-accumulate ---
            dwsq = consts.tile([128, BPC * 48], f32)
            nc.scalar.activation(
                out=dwsq[:], in_=dwt[:], func=Act.Square,
                accum_out=partials[:, 6:7],
            )

            ident = consts.tile([128, 128], bf16)
            make_identity(nc, ident)

            # split-K ramp operands: ones weights at block-aligned rows and
            # a copy of batch-0's |y|^2 hi/lo rows at partitions 0/1, 64/65.
            ones2 = consts.tile([128, 128], bf16)
            nc.vector.memset(ones2, 1.0)
            yp2 = consts.tile([128, P2], bf16)
            for q in range(2):
                e_a.dma_start(out=yp2[64 * q:64 * q + 1, :], in_=nyh0[:])
                e_a.dma_start(out=yp2[64 * q + 1:64 * q + 2, :], in_=nyl0[:])

            # ---------- main distance + min pipeline ----------
            colaccs = [
                opnds.tile([128, P2], bf16, tag="colacc", name=f"colacc{b}")
                for b in range(BPC)
            ]
            rowpartss = [
                opnds.tile([128, NT, 128], bf16, tag="rowparts",
                           name=f"rowparts{b}")
                for b in range(BPC)
            ]

            def emit_seg(b, t0, n, split_k=False):
                """n i-tiles [t0, t0+n): matmuls + evac + DVE mins.

                split_k: issue the K=9 product rows immediately and
                accumulate the two |y|^2 rows in a second matmul pass, so
                the PE starts before the norm-row DMAs land (startup ramp).
                """
                xp, yp, nrmx = xps[b], yps[b], nrmxs[b]
                colacc, rowparts = colaccs[b], rowpartss[b]
                s4 = scr.tile([128, n, P2], bf16, tag=f"s4_{n}", bufs=3,
                              name=f"s4_{b}_{t0}")
                for u in range(n):
                    t = t0 + u
                    # PE row block alternates per pair; rank = col chunk
                    qb = 64 * ((t >> 1) & 1)
                    r = (t >> 2) * 2 + (t & 1)
                    lsl = xp[qb:qb + KROWS, 128 * r:128 * (r + 1)]
                    g = ps.tile([128, P2], f32, tag="dgrp", name=f"g_{b}_{t}")
                    for c in range(NJ):
                        sl = slice(c * 512, (c + 1) * 512)
                        if split_k:
                            nc.tensor.matmul(
                                g[:, sl], lsl[0:9, :], yp[qb:qb + 9, sl],
                                tile_position=(qb, 0), start=True, stop=False,
                                skip_group_check=True,
                            )
                        else:
                            nc.tensor.matmul(
                                g[:, sl], lsl, yp[qb:qb + KROWS, sl],
                                tile_position=(qb, 0),
                            )
                    if split_k:
                        for c in range(NJ):
                            sl = slice(c * 512, (c + 1) * 512)
                            nc.tensor.matmul(
                                g[:, sl], ones2[qb:qb + 2, 0:128],
                                yp2[qb:qb + 2, sl],
                                tile_position=(qb, 0), start=False, stop=True,
                                skip_group_check=True,
                            )
                    # ACT evacuation: D = E + |x|^2 (partition bias), bf16
                    nc.scalar.activation(
                        out=s4[:, u, :], in_=g[:], func=Act.Identity,
                        bias=nrmx[:, t:t + 1],
                    )
                # row-min fold chain, seg-batched (DVE bf16 2x)
                src = s4[:]
                w = P2
                while w > 256:
                    w //= 2
                    dst = scr.tile([128, n, w], bf16, tag=f"rp{w}_{n}", bufs=2,
                                   name=f"rp{w}_{b}_{t0}")
                    nc.vector.tensor_tensor(
                        out=dst[:], in0=src[:, :, 0:w], in1=src[:, :, w:2 * w],
                        op=Alu.min,
                    )
                    src = dst[:]
                nc.vector.tensor_tensor(
                    out=rowparts[:, t0:t0 + n, :],
                    in0=src[:, :, 0:128], in1=src[:, :, 128:256], op=Alu.min,
                )
                # col-min accumulate, seg-batched
                if n == 4:
                    m2 = scr.tile([128, 2, P2], bf16, tag="m2", bufs=2)
                    nc.vector.tensor_tensor(
                        out=m2[:], in0=s4[:, 0:2, :], in1=s4[:, 2:4, :],
                        op=Alu.min,
                    )
                    lo, hi = m2[:, 0, :], m2[:, 1, :]
                elif n == 2:
                    lo, hi = s4[:, 0, :], s4[:, 1, :]
                else:
                    lo, hi = s4[:, 0, :], None
                if t0 == 0:
                    if hi is None:
                        nc.vector.tensor_copy(out=colacc[:], in_=lo)
                    else:
                        nc.vector.tensor_tensor(
                            out=colacc[:], in0=lo, in1=hi, op=Alu.min
                        )
                else:
                    if hi is None:
                        nc.vector.tensor_tensor(
                            out=colacc[:], in0=colacc[:], in1=lo, op=Alu.min
                        )
                    else:
                        mmp = scr.tile([128, P2], bf16, tag="mmp", bufs=2,
                                       name=f"mmp_{b}_{t0}")
                        nc.vector.tensor_tensor(out=mmp[:], in0=lo, in1=hi,
                                                op=Alu.min)
                        nc.vector.tensor_tensor(
                            out=colacc[:], in0=colacc[:], in1=mmp[:],
                            op=Alu.min,
                        )

            def emit_tail(b):
                """Row-min finish + cham_v transposes + masked sums."""
                colacc, rowparts, mk = colaccs[b], rowpartss[b], mks[b]
                # finish row-min: [128,16,128] -> [128,16] min, then sum
                rowacc = small.tile([128, NT], f32, tag="rowacc")
                nc.vector.tensor_reduce(
                    out=rowacc[:], in_=rowparts[:], axis=X, op=Alu.min,
                )
                nc.vector.tensor_reduce(
                    out=partials[:, 2 * b + 1:2 * b + 2], in_=rowacc[:],
                    axis=X, op=Alu.add,
                )
                # cham_v: fold colacc partitions via PE transpose; ACT
                # evacuates the bf16 transpose out of PSUM (contiguous read)
                # so the DVE reduce runs on SBUF without PSUM seg penalties.
                chamv = small.tile([128, NC128], f32, tag="chamv")
                for r in range(2):
                    tp = ps.tile([128, 1024], bf16, tag="dgrp")
                    for cc in range(8):
                        cidx = r * 8 + cc
                        nc.tensor.transpose(
                            tp[:, cc * 128:(cc + 1) * 128],
                            colacc[:, cidx * 128:(cidx + 1) * 128],
                            ident[:],
                        )
                    tps = small.tile([128, 1024], bf16, tag="tps", bufs=2)
                    nc.scalar.copy(tps[:], tp[:])
                    nc.vector.tensor_reduce(
                        out=chamv[:, r * 8:(r + 1) * 8],
                        in_=tps[:].rearrange("p (a c) -> p a c", c=128),
                        axis=X, op=Alu.min,
                    )
                # masked sum: out = (chamv + 0) * mask, accum = sum
                prod = small.tile([128, NC128], f32, tag="prod")
                nc.vector.scalar_tensor_tensor(
                    out=prod[:], in0=chamv[:], scalar=0.0, in1=mk[:],
                    op0=Alu.add, op1=Alu.mult,
                    accum_out=partials[:, 2 * b:2 * b + 1],
                )

            # batch 0 ramps in with split-K singles (PE starts before the
            # norm rows land), then pairs/quads; batch-0 tail is emitted
            # after batch 1's first quad so it fills pipeline gaps instead
            # of serializing the batch hand-off.
            emit_seg(0, 0, 1, split_k=True)
            emit_seg(0, 1, 1, split_k=True)
            emit_seg(0, 2, 2)
            for t0 in (4, 8, 12):
                emit_seg(0, t0, 4)
            emit_seg(1, 0, 4)
            emit_tail(0)
            for t0 in (4, 8, 12):
                emit_seg(1, t0, 4)
            emit_tail(1)

            # ---------- cross-partition sum of all partials via PE ----------
            fin = ps.tile([128, 512], f32, tag="dgrp")
            nc.tensor.matmul(fin[0:1, 0:8], ones128[:], partials[:])
            res = small.tile([1, 8], f32, tag="res")
            nc.scalar.copy(res[:], fin[0:1, 0:8])
            nc.sync.dma_start(out=out_h[:], in_=res[:])

    nc.compile()
    return nc


def get_compiled():
    if "nc" not in _CACHE:
        _CACHE["nc"] = build_bass()
    return _CACHE["nc"]


def make_in_maps(v, v_pred, mask, pred_dw):
    import ml_dtypes

    bf16 = ml_dtypes.bfloat16
    v = np.asarray(v, np.float32)
    v_pred = np.asarray(v_pred, np.float32)
    mask = np.asarray(mask, np.float32)
    pred_dw = np.asarray(pred_dw, np.float32)

    # lossless bf16 hi/lo repacking of the matmul operands
    wT = (-2.0 * v_pred).transpose(0, 2, 1)           # (16, 3, 2048) f32
    wh = wT.astype(bf16)
    wl = (wT - wh.astype(np.float32)).astype(bf16)
    ones = np.ones((B, 2, P1), dtype=bf16)
    # lhsT rows: [wh x3, wl x3, wh x3, 1, 1]  (K=11)
    xprod = np.concatenate([wh, wl, wh, ones], axis=1)      # (16, 11, 2048)
    # PE row block q' = (t>>1)&1, rank r = (t>>2)*2 + (t&1):
    # xprod_dev[b, q', :, 128r:128(r+1)] = xprod[b, :, 128t:128(t+1)]
    # column c = 128t + j with t = 4a + 2b2 + b1 -> (a, b2, b1, j);
    # block = b2, rank = 2a + b1.
    xprod_dev = np.ascontiguousarray(
        xprod.reshape(B, KROWS, 4, 2, 2, 128).transpose(0, 3, 1, 2, 4, 5)
        .reshape(B, 2, KROWS, 1024)
    )
    yT = v.transpose(0, 2, 1)
    yh = yT.astype(bf16)
    yl = (yT - yh.astype(np.float32)).astype(bf16)
    zeros2 = np.zeros((B, 2, P2), dtype=bf16)
    # rhs rows: [yh x3, yh x3, yl x3, <y2 hi/lo slots>]  (K=11)
    yprod = np.concatenate([yh, yh, yl, zeros2], axis=1)    # (16, 11, 2048)

    # cdx permuted so nrmx[p, t] = |x_{128t+p}|^2
    cdx = np.ascontiguousarray(
        v_pred.reshape(B, 16, 128, 3).transpose(0, 2, 1, 3).reshape(B, 128, 48)
    )
    # cdy as-is: nrmy[p, n] = |y_{16p+n}|^2 -> row-major j ordering
    cdy = v.reshape(B, 128, 48)
    mask_flat = mask.reshape(B, P2)
    # maskT[b, p, c] = mask_flat[b, c*128 + p]
    maskT = np.ascontiguousarray(
        mask_flat.reshape(B, NC128, 128).transpose(0, 2, 1)
    )
    in_maps = []
    for kcore in range(NCORES):
        b0 = BPC * kcore
        dwp = np.concatenate(
            [pred_dw[b0 + i].reshape(128, 48) for i in range(BPC)], axis=1
        )
        in_maps.append({
            "xprod": np.ascontiguousarray(xprod_dev[b0:b0 + BPC]),
            "yprod": np.ascontiguousarray(yprod[b0:b0 + BPC]),
            "cdx": np.ascontiguousarray(cdx[b0:b0 + BPC]),
            "cdy": np.ascontiguousarray(cdy[b0:b0 + BPC]),
            "maskT": np.ascontiguousarray(maskT[b0:b0 + BPC]),
            "dw": np.ascontiguousarray(dwp),
        })
    return in_maps


def combine_outs(outs):
    """outs: (8, 8) array of per-core partial rows -> (loss, loss_normals)."""
    outs = np.asarray(outs, np.float64)
    mcols = [2 * i for i in range(BPC)]
    rcols = [2 * i + 1 for i in range(BPC)]
    msum = outs[:, mcols].sum()
    rsum = outs[:, rcols].sum()
    dsum = outs[:, 6].sum()
    loss = msum / (B * P2) + rsum / (B * P1) + dsum / (B * P1 * D)
    return (np.float32(loss), np.float32(0.0))


def kernel(**inputs):
    from concourse.bass_utils import run_bass_kernel_spmd

    nc = get_compiled()
    in_maps = make_in_maps(
        inputs["v"], inputs["v_pred"], inputs["mask"], inputs["pred_dw"]
    )
    res = run_bass_kernel_spmd(nc, in_maps, core_ids=list(range(NCORES)))
    outs = np.stack([r["out"].reshape(8) for r in res.results])
    return combine_outs(outs)


# revision 26
# speedup vs baseline: 1.0739x; 1.0086x over previous
"""Chamfer-distance loss (CCHLoss) kernel for 8 Trainium2 NeuronCores.

Contract: kernel(**inputs) takes the FULL unsharded inputs
  v:        (16, 2048, 3) f32
  v_pred:   (16, 2048, 3) f32
  mask:     (4, 4, 2, 32, 32) f32
  pred_dw:  (16, 2048, 3) f32
and returns (loss, loss_normals) matching reference().

Strategy: data-parallel over the B=16 batch dim, 2 batches per core.
Per batch the 2048x2048 squared-distance matrix is produced by TensorE
as E = |y|^2 - 2 x.y via a K=11 bf16 hi/lo matmul; the |x|^2 term is
fused into the ACT-engine PSUM->SBUF evacuation as a per-partition
fp32 bias (activation Identity), so ACT emits D = E + |x|^2 in bf16.
i-tiles rotate over the four 32-row PE groups (tile_position derived
from base partitions 0/32/64/96; rhs replicated into the four blocks)
so LDWEIGHTS overlaps the matmul stream.  Reductions are split:
  - VectorE: col-min accumulate (bf16 2x tensor_tensor) + first
    row-min fold per group,
  - GpSimd (Pool): remaining row-min fold tiers for most groups,
  - tail: fold chains + PE transposes + a fused scalar_tensor_tensor
    masked sum.
Host only shards/permutes inputs and sums 8 cores' partial sums.
"""

import numpy as np

B, P1, P2, D = 16, 2048, 2048, 3
NCORES = 8
BPC = B // NCORES  # batches per core
NT = P1 // 128     # i-tiles per batch
NJ = P2 // 512     # matmul j-chunks per group
NC128 = P2 // 128  # 128-wide j-chunks (transpose fold)
KROWS = 11         # lhsT/rhs contraction rows

_CACHE = {}


def build_bass():
    """Build + compile the per-core Bass program (same program all 8 cores)."""
    import concourse.bacc as bacc
    import concourse.tile as tile
    from concourse import mybir
    from concourse.masks import make_identity

    f32 = mybir.dt.float32
    bf16 = mybir.dt.bfloat16
    Alu = mybir.AluOpType
    Act = mybir.ActivationFunctionType
    X = mybir.AxisListType.X

    nc = bacc.Bacc("TRN2", target_bir_lowering=False, debug=False)

    # host-shipped tensors
    xprod_h = nc.dram_tensor("xprod", (BPC, 2, KROWS, 1024), bf16, kind="ExternalInput")
    yprod_h = nc.dram_tensor("yprod", (BPC, KROWS, P2), bf16, kind="ExternalInput")
    cdx_h = nc.dram_tensor("cdx", (BPC, 128, 48), f32, kind="ExternalInput")
    cdy_h = nc.dram_tensor("cdy", (BPC, 128, 48), f32, kind="ExternalInput")
    maskT_h = nc.dram_tensor("maskT", (BPC, 128, NC128), f32, kind="ExternalInput")
    dw_h = nc.dram_tensor("dw", (128, BPC * 48), f32, kind="ExternalInput")
    out_h = nc.dram_tensor("out", (1, 8), f32, kind="ExternalOutput")

    with tile.TileContext(nc) as tc:
        with (
            tc.tile_pool(name="consts", bufs=1) as consts,
            tc.tile_pool(name="opnds", bufs=2) as opnds,
            tc.tile_pool(name="scr", bufs=3) as scr,
            tc.tile_pool(name="small", bufs=4) as small,
            tc.tile_pool(name="ps", bufs=2, space="PSUM") as ps,
        ):
            ones128 = consts.tile([128, 1], f32)
            nc.vector.memset(ones128, 1.0)
            partials = consts.tile([128, 8], f32)
            nc.vector.memset(partials, 0.0)

            # ---------- input DMAs first (all of them) ----------
            # queues: sync = yp input/replica + norm-row writes (critical);
            # scalar = tiny cd inputs then norm rows, NO dmas in steady state;
            # gpsimd (SWDGE) = xp + non-critical loads.
            e_a = nc.sync
            e_b = nc.scalar
            e_g = nc.gpsimd
            cds, yps, xps, mks = [], [], [], []
            for b in range(BPC):
                cd = opnds.tile([128, 96], f32, tag="cd")
                e_b.dma_start(out=cd[:, 0:48], in_=cdy_h[b])
                e_b.dma_start(out=cd[:, 48:96], in_=cdx_h[b])
                cds.append(cd)
            for b in range(BPC):
                yp = opnds.tile([128, P2], bf16, tag="yp")
                e_a.dma_start(out=yp[0:KROWS, :], in_=yprod_h[b])
                # replicate rows 0..8 into block 2 (norm rows come later)
                e_a.dma_start(out=yp[64:64 + 9, :], in_=yp[0:9, :])
                yps.append(yp)
                xp = opnds.tile([128, 1024], bf16, tag="xp")
                for q in range(2):
                    e_g.dma_start(
                        out=xp[64 * q:64 * q + KROWS, :], in_=xprod_h[b, q]
                    )
                xps.append(xp)
                mk = small.tile([128, NC128], f32, tag="mk")
                e_g.dma_start(out=mk[:], in_=maskT_h[b])
                mks.append(mk)
            dwt = consts.tile([128, BPC * 48], f32)
            e_g.dma_start(out=dwt[:], in_=dw_h[:])

            # warm the ACT activation table off the critical path
            warm = consts.tile([1, 1], f32)
            nc.vector.memset(warm, 0.0)
            warmo = consts.tile([1, 1], f32)
            nc.scalar.activation(out=warmo[:], in_=warm[:], func=Act.Square)

            # ---------- per-batch norm prep ----------
            # one square + one reduce covers both y (cols 0:16) and x (16:32)
            nrmxs = []
            nyh0 = nyl0 = None
            for b in range(BPC):
                cd = cds[b]
                yp = yps[b]
                sq = opnds.tile([128, 96], f32, tag="sq")
                nc.scalar.activation(out=sq[:], in_=cd[:], func=Act.Square)
                nrm = opnds.tile([128, 32], f32, tag="nrm")
                nc.vector.tensor_reduce(
                    out=nrm[:], in_=sq[:].rearrange("p (n d) -> p n d", d=3),
                    axis=X, op=Alu.add,
                )
                # y norms: bf16 hi/lo rows into both blocks (DVE-only chain)
                nyh = opnds.tile([128, 16], bf16, tag="nyh")
                nc.vector.tensor_copy(out=nyh[:], in_=nrm[:, 0:16])
                nyl = opnds.tile([128, 16], bf16, tag="nyl")
                nc.vector.tensor_tensor(
                    out=nyl[:], in0=nrm[:, 0:16], in1=nyh[:], op=Alu.subtract
                )
                for q in range(2):
                    e_b.dma_start(out=yp[64 * q + 9:64 * q + 10, :], in_=nyh[:])
                    e_b.dma_start(out=yp[64 * q + 10:64 * q + 11, :], in_=nyl[:])
                # x norms: nrm[:, 16:32] stays fp32, consumed as ACT bias
                nrmxs.append(nrm[:, 16:32])
                if b == 0:
                    nyh0, nyl0 = nyh, nyl

            # --- mean(pred_dw^2) partial: ACT square with sum-accumulate ---
            dwsq = consts.tile([128, BPC * 48], f32)
            nc.scalar.activation(
                out=dwsq[:], in_=dwt[:], func=Act.Square,
                accum_out=partials[:, 6:7],
            )

            ident = consts.tile([128, 128], bf16)
            make_identity(nc, ident)

            # split-K ramp operands: ones weights at block-aligned rows and
            # a copy of batch-0's |y|^2 hi/lo rows at partitions 0/1, 64/65.
            ones2 = consts.tile([128, 128], bf16)
            nc.vector.memset(ones2, 1.0)
            yp2 = consts.tile([128, P2], bf16)
            for q in range(2):
                e_a.dma_start(out=yp2[64 * q:64 * q + 1, :], in_=nyh0[:])
                e_a.dma_start(out=yp2[64 * q + 1:64 * q + 2, :], in_=nyl0[:])

            # ---------- main distance + min pipeline ----------
            colaccs = [
                opnds.tile([128, P2], bf16, tag="colacc", name=f"colacc{b}")
                for b in range(BPC)
            ]
            rowpartss = [
                opnds.tile([128, NT, 128], bf16, tag="rowparts",
                           name=f"rowparts{b}")
                for b in range(BPC)
            ]

            def emit_seg(b, t0, n, split_k=False):
                """n i-tiles [t0, t0+n): matmuls + evac + DVE mins.

                split_k: issue the K=9 product rows immediately and
                accumulate the two |y|^2 rows in a second matmul pass, so
                the PE starts before the norm-row DMAs land (startup ramp).
                """
                xp, yp, nrmx = xps[b], yps[b], nrmxs[b]
                colacc, rowparts = colaccs[b], rowpartss[b]
                s4 = scr.tile([128, n, P2], bf16, tag=f"s4_{n}", bufs=3,
                              name=f"s4_{b}_{t0}")
                for u in range(n):
                    t = t0 + u
                    # PE row block alternates per pair; rank = col chunk
                    qb = 64 * ((t >> 1) & 1)
                    r = (t >> 2) * 2 + (t & 1)
                    lsl = xp[qb:qb + KROWS, 128 * r:128 * (r + 1)]
                    g = ps.tile([128, P2], f32, tag="dgrp", name=f"g_{b}_{t}")
                    for c in range(NJ):
                        sl = slice(c * 512, (c + 1) * 512)
                        if split_k:
                            nc.tensor.matmul(
                                g[:, sl], lsl[0:9, :], yp[qb:qb + 9, sl],
                                tile_position=(qb, 0), start=True, stop=False,
                                skip_group_check=True,
                            )
                        else:
                            nc.tensor.matmul(
                                g[:, sl], lsl, yp[qb:qb + KROWS, sl],
                                tile_position=(qb, 0),
                            )
                    if split_k:
                        for c in range(NJ):
                            sl = slice(c * 512, (c + 1) * 512)
                            nc.tensor.matmul(
                                g[:, sl], ones2[qb:qb + 2, 0:128],
                                yp2[qb:qb + 2, sl],
                                tile_position=(qb, 0), start=False, stop=True,
                                skip_group_check=True,
                            )
                    # ACT evacuation: D = E + |x|^2 (partition bias), bf16
                    nc.scalar.activation(
                        out=s4[:, u, :], in_=g[:], func=Act.Identity,
                        bias=nrmx[:, t:t + 1],
                    )
                # row-min fold chain, seg-batched (DVE bf16 2x)
                src = s4[:]
                w = P2
                while w > 256:
                    w //= 2
                    dst = scr.tile([128, n, w], bf16, tag=f"rp{w}_{n}", bufs=2,
                                   name=f"rp{w}_{b}_{t0}")
                    nc.vector.tensor_tensor(
                        out=dst[:], in0=src[:, :, 0:w], in1=src[:, :, w:2 * w],
                        op=Alu.min,
                    )
                    src = dst[:]
                nc.vector.tensor_tensor(
                    out=rowparts[:, t0:t0 + n, :],
                    in0=src[:, :, 0:128], in1=src[:, :, 128:256], op=Alu.min,
                )
                # col-min accumulate, seg-batched
                if n == 4:
                    m2 = scr.tile([128, 2, P2], bf16, tag="m2", bufs=2)
                    nc.vector.tensor_tensor(
                        out=m2[:], in0=s4[:, 0:2, :], in1=s4[:, 2:4, :],
                        op=Alu.min,
                    )
                    lo, hi = m2[:, 0, :], m2[:, 1, :]
                elif n == 2:
                    lo, hi = s4[:, 0, :], s4[:, 1, :]
                else:
                    lo, hi = s4[:, 0, :], None
                if t0 == 0:
                    if hi is None:
                        nc.vector.tensor_copy(out=colacc[:], in_=lo)
                    else:
                        nc.vector.tensor_tensor(
                            out=colacc[:], in0=lo, in1=hi, op=Alu.min
                        )
                else:
                    if hi is None:
                        nc.vector.tensor_tensor(
                            out=colacc[:], in0=colacc[:], in1=lo, op=Alu.min
                        )
                    else:
                        mmp = scr.tile([128, P2], bf16, tag="mmp", bufs=2,
                                       name=f"mmp_{b}_{t0}")
                        nc.vector.tensor_tensor(out=mmp[:], in0=lo, in1=hi,
                                                op=Alu.min)
                        nc.vector.tensor_tensor(
                            out=colacc[:], in0=colacc[:], in1=mmp[:],
                            op=Alu.min,
                        )

            def emit_tail(b):
                """Row-min finish + cham_v transposes + masked sums."""
                colacc, rowparts, mk = colaccs[b], rowpartss[b], mks[b]
                # finish row-min: [128,16,128] -> [128,16] min, then sum
                rowacc = small.tile([128, NT], f32, tag="rowacc")
                nc.vector.tensor_reduce(
                    out=rowacc[:], in_=rowparts[:], axis=X, op=Alu.min,
                )
                nc.vector.tensor_reduce(
                    out=partials[:, 2 * b + 1:2 * b + 2], in_=rowacc[:],
                    axis=X, op=Alu.add,
                )
                # cham_v: fold colacc partitions via PE transpose; ACT
                # evacuates the bf16 transpose out of PSUM (contiguous read)
                # so the DVE reduce runs on SBUF without PSUM seg penalties.
                chamv = small.tile([128, NC128], f32, tag="chamv")
                for r in range(2):
                    tp = ps.tile([128, 1024], bf16, tag="dgrp")
                    for cc in range(8):
                        cidx = r * 8 + cc
                        nc.tensor.transpose(
                            tp[:, cc * 128:(cc + 1) * 128],
                            colacc[:, cidx * 128:(cidx + 1) * 128],
                            ident[:],
                        )
                    tps = small.tile([128, 1024], bf16, tag="tps", bufs=2)
                    nc.scalar.copy(tps[:], tp[:])
                    nc.vector.tensor_reduce(
                        out=chamv[:, r * 8:(r + 1) * 8],
                        in_=tps[:].rearrange("p (a c) -> p a c", c=128),
                        axis=X, op=Alu.min,
                    )
                # masked sum: out = (chamv + 0) * mask, accum = sum
                prod = small.tile([128, NC128], f32, tag="prod")
                nc.vector.scalar_tensor_tensor(
                    out=prod[:], in0=chamv[:], scalar=0.0, in1=mk[:],
                    op0=Alu.add, op1=Alu.mult,
                    accum_out=partials[:, 2 * b:2 * b + 1],
                )

            # batch 0 ramps in with split-K singles (PE starts before the
            # norm rows land), then pairs/quads; batch-0 tail is emitted
            # after batch 1's first quad so it fills pipeline gaps instead
            # of serializing the batch hand-off.
            emit_seg(0, 0, 2, split_k=True)
            for t0 in (2, 4, 6, 8, 10, 12, 14):
                emit_seg(0, t0, 2)
            emit_seg(1, 0, 4)
            emit_tail(0)
            for t0 in (4, 8, 12):
                emit_seg(1, t0, 4)
            emit_tail(1)

            # ---------- cross-partition sum of all partials via PE ----------
            fin = ps.tile([128, 512], f32, tag="dgrp")
            nc.tensor.matmul(fin[0:1, 0:8], ones128[:], partials[:])
            res = small.tile([1, 8], f32, tag="res")
            nc.scalar.copy(res[:], fin[0:1, 0:8])
            nc.sync.dma_start(out=out_h[:], in_=res[:])

    nc.compile()
    return nc


def get_compiled():
    if "nc" not in _CACHE:
        _CACHE["nc"] = build_bass()
    return _CACHE["nc"]


def make_in_maps(v, v_pred, mask, pred_dw):
    import ml_dtypes

    bf16 = ml_dtypes.bfloat16
    v = np.asarray(v, np.float32)
    v_pred = np.asarray(v_pred, np.float32)
    mask = np.asarray(mask, np.float32)
    pred_dw = np.asarray(pred_dw, np.float32)

    # lossless bf16 hi/lo repacking of the matmul operands
    wT = (-2.0 * v_pred).transpose(0, 2, 1)           # (16, 3, 2048) f32
    wh = wT.astype(bf16)
    wl = (wT - wh.astype(np.float32)).astype(bf16)
    ones = np.ones((B, 2, P1), dtype=bf16)
    # lhsT rows: [wh x3, wl x3, wh x3, 1, 1]  (K=11)
    xprod = np.concatenate([wh, wl, wh, ones], axis=1)      # (16, 11, 2048)
    # PE row block q' = (t>>1)&1, rank r = (t>>2)*2 + (t&1):
    # xprod_dev[b, q', :, 128r:128(r+1)] = xprod[b, :, 128t:128(t+1)]
    # column c = 128t + j with t = 4a + 2b2 + b1 -> (a, b2, b1, j);
    # block = b2, rank = 2a + b1.
    xprod_dev = np.ascontiguousarray(
        xprod.reshape(B, KROWS, 4, 2, 2, 128).transpose(0, 3, 1, 2, 4, 5)
        .reshape(B, 2, KROWS, 1024)
    )
    yT = v.transpose(0, 2, 1)
    yh = yT.astype(bf16)
    yl = (yT - yh.astype(np.float32)).astype(bf16)
    zeros2 = np.zeros((B, 2, P2), dtype=bf16)
    # rhs rows: [yh x3, yh x3, yl x3, <y2 hi/lo slots>]  (K=11)
    yprod = np.concatenate([yh, yh, yl, zeros2], axis=1)    # (16, 11, 2048)

    # cdx permuted so nrmx[p, t] = |x_{128t+p}|^2
    cdx = np.ascontiguousarray(
        v_pred.reshape(B, 16, 128, 3).transpose(0, 2, 1, 3).reshape(B, 128, 48)
    )
    # cdy as-is: nrmy[p, n] = |y_{16p+n}|^2 -> row-major j ordering
    cdy = v.reshape(B, 128, 48)
    mask_flat = mask.reshape(B, P2)
    # maskT[b, p, c] = mask_flat[b, c*128 + p]
    maskT = np.ascontiguousarray(
        mask_flat.reshape(B, NC128, 128).transpose(0, 2, 1)
    )
    in_maps = []
    for kcore in range(NCORES):
        b0 = BPC * kcore
        dwp = np.concatenate(
            [pred_dw[b0 + i].reshape(128, 48) for i in range(BPC)], axis=1
        )
        in_maps.append({
            "xprod": np.ascontiguousarray(xprod_dev[b0:b0 + BPC]),
            "yprod": np.ascontiguousarray(yprod[b0:b0 + BPC]),
            "cdx": np.ascontiguousarray(cdx[b0:b0 + BPC]),
            "cdy": np.ascontiguousarray(cdy[b0:b0 + BPC]),
            "maskT": np.ascontiguousarray(maskT[b0:b0 + BPC]),
            "dw": np.ascontiguousarray(dwp),
        })
    return in_maps


def combine_outs(outs):
    """outs: (8, 8) array of per-core partial rows -> (loss, loss_normals)."""
    outs = np.asarray(outs, np.float64)
    mcols = [2 * i for i in range(BPC)]
    rcols = [2 * i + 1 for i in range(BPC)]
    msum = outs[:, mcols].sum()
    rsum = outs[:, rcols].sum()
    dsum = outs[:, 6].sum()
    loss = msum / (B * P2) + rsum / (B * P1) + dsum / (B * P1 * D)
    return (np.float32(loss), np.float32(0.0))


def kernel(**inputs):
    from concourse.bass_utils import run_bass_kernel_spmd

    nc = get_compiled()
    in_maps = make_in_maps(
        inputs["v"], inputs["v_pred"], inputs["mask"], inputs["pred_dw"]
    )
    res = run_bass_kernel_spmd(nc, in_maps, core_ids=list(range(NCORES)))
    outs = np.stack([r["out"].reshape(8) for r in res.results])
    return combine_outs(outs)
